# revision 1
# baseline (speedup 1.0000x reference)
"""EvidenceLevelAttention (additive attention GNN message passing) on 8 trn2 cores.

Math per batch b (B=8, N=256, H=300):
    ai = h @ W0a.T ; aj = h @ W0b.T                     (W0a = W0[:, :H], W0b = W0[:, H:])
    p[i, j] = w1 . relu(ai[i] + aj[j] + b0)  (+ b1, dropped: softmax shift-invariant)
    a = softmax(p, axis=-1) ;  y = a @ h

Data-parallel: core c computes batch c. Heavy math in fp16 with fp32 PSUM
accumulation.

Layout: hidden dim k (300 -> padded 384 = 3x128) on partitions for the pairwise
phase, so the per-i bias (aiT[:, i] + b0) is a per-partition scalar: one fused
DVE tensor_scalar(add, max) per (i, k-block) computes relu(ajT + bias) for all
256 j. TensorE then contracts with w1 by loading T as the stationary operand
(128 j columns at a time) and streaming w1 as the 1-wide moving operand, so
p^T[j, i] accumulates as full 128-partition psum columns. Softmax needs no
transposes: p is O(1) here so exp(p) is computed without max-subtraction, row
sums come from a ones-matmul, and 1/s is applied as a per-partition scale on
the final output u = e^T.T @ h.
"""

import numpy as np

import concourse.bass as bass
import concourse.mybir as mybir
import concourse.tile as tile
from concourse import bacc
from concourse.bass_utils import run_bass_kernel_spmd
from concourse.masks import make_identity

B, N, H = 8, 256, 300
HB = 3          # hidden-dim blocks of 128
HP = HB * 128   # padded hidden dim
NB = 2          # row blocks of 128
F32 = mybir.dt.float32
F16 = mybir.dt.float16
ACT_EVERY = 3   # legacy knob (unused when ENGINE_PATTERN set)
ENGINE_PATTERN = ["V", "A", "V", "V", "G", "V"]  # full-block relu engine rotation
TAIL_PATTERN = ["G", "V", "A", "V", "V", "A"]  # tail-op rotation (ttt is its own tile, so a different engine keeps single-producer tiles)
N_I = N         # phase-B iteration count (reduced for calibration benches)
SKIP_RELU = False   # timing-only: single-op tensor_scalar (wrong math)
SKIP_MM = False     # timing-only: skip phase-B matmuls (wrong math)
FD_TEST = None      # timing-only: shrink elementwise free dim (wrong math)
T_BUFS = 24

_CACHE = {}


def _emit(nc):
    f32, f16 = F32, F16
    Alu = mybir.AluOpType
    Relu = mybir.ActivationFunctionType.Relu
    Exp = mybir.ActivationFunctionType.Exp

    h_in = nc.dram_tensor("h", [N, H], f32, kind="ExternalInput")
    w0_in = nc.dram_tensor("w0", [H, 2 * H], f32, kind="ExternalInput")
    b0_in = nc.dram_tensor("b0", [H], f32, kind="ExternalInput")
    w1_in = nc.dram_tensor("w1", [H], f32, kind="ExternalInput")
    y_out = nc.dram_tensor("y", [N, H], f32, kind="ExternalOutput")

    with tile.TileContext(nc) as tc:
        with (
            tc.tile_pool(name="const", bufs=1) as const,
            tc.tile_pool(name="work", bufs=2) as work,
            tc.tile_pool(name="tpool", bufs=T_BUFS) as tpool,
            tc.tile_pool(name="psA", bufs=2, space="PSUM") as psA,
            tc.tile_pool(name="psT", bufs=2, space="PSUM") as psT,
            tc.tile_pool(name="psP", bufs=1, space="PSUM") as psP,
            tc.tile_pool(name="psO", bufs=2, space="PSUM") as psO,
        ):
            # ---------------- phase 0: loads, casts, transposes ----------------
            # h rows, fp32 then fp16 (k-padded with zeros)
            h_f32 = [const.tile([128, H], f32, name=f"h_f32_{k}") for k in range(NB)]
            h_f16 = [const.tile([128, HP], f16, name=f"h_f16_{k}") for k in range(NB)]
            for ib in range(NB):
                nc.sync.dma_start(out=h_f32[ib], in_=h_in[ib * 128:(ib + 1) * 128, :])
                nc.vector.memset(h_f16[ib][:, H:HP], 0.0)
                nc.vector.memset(h_f16[ib][:, H:H + 1], 1.0)  # ones col for fused row-sum
                nc.vector.tensor_scalar(out=h_f16[ib][:, 0:H], in0=h_f32[ib], scalar1=0.0, scalar2=None, op0=Alu.add)

            # hT[hb]: [128 h, 256 n]  (PE transpose of fp16 tiles)
            ident = const.tile([128, 128], f16)
            make_identity(nc, ident)
            hT = [const.tile([128, N], f16, name=f"hT_{k}") for k in range(HB)]
            ncopy = 0
            for hb in range(HB):
                for ib in range(NB):
                    pst = psT.tile([128, 128], f16, tag="tr")
                    nc.tensor.transpose(
                        pst, h_f16[ib][:, hb * 128:(hb + 1) * 128], ident,
                    )
                    dst_sl = hT[hb][:, ib * 128:(ib + 1) * 128]
                    if ncopy % 2 == 0:
                        nc.vector.tensor_scalar(out=dst_sl, in0=pst, scalar1=0.0, scalar2=None, op0=Alu.add)
                    else:
                        nc.scalar.copy(dst_sl, pst)
                    ncopy += 1

            # W0, k-blocked rows, columns split [W0a | pad | W0b | pad], fp16
            w0_f16 = []
            for kb in range(HB):
                k0 = kb * 128
                ksz = min(H, k0 + 128) - k0
                t32 = work.tile([128, 2 * H], f32, tag="w0scratch")
                tf = const.tile([128, 2 * HP], f16, name=f"w0f16_{kb}")
                nc.sync.dma_start(out=t32[0:ksz, :], in_=w0_in[k0:k0 + ksz, :])
                nc.vector.memset(tf, 0.0)
                nc.vector.tensor_scalar(out=tf[0:ksz, 0:H], in0=t32[0:ksz, 0:H], scalar1=0.0, scalar2=None, op0=Alu.add)
                nc.vector.tensor_scalar(out=tf[0:ksz, HP:HP + H], in0=t32[0:ksz, H:2 * H], scalar1=0.0, scalar2=None, op0=Alu.add)
                w0_f16.append(tf)

            # W0aT/W0bT[hb]: [128 h, 384 k] via PE transpose (128x128 blocks)
            w0aT = [const.tile([128, HP], f16, name=f"w0aT_{k}") for k in range(HB)]
            w0bT = [const.tile([128, HP], f16, name=f"w0bT_{k}") for k in range(HB)]
            for half, dst in ((0, w0aT), (1, w0bT)):
                for hb in range(HB):
                    for kb in range(HB):
                        pst = psT.tile([128, 128], f16, tag="tr")
                        nc.tensor.transpose(
                            pst,
                            w0_f16[kb][:, half * HP + hb * 128: half * HP + (hb + 1) * 128],
                            ident,
                        )
                        dst_sl = dst[hb][:, kb * 128:(kb + 1) * 128]
                        if ncopy % 2 == 0:
                            nc.vector.tensor_scalar(out=dst_sl, in0=pst, scalar1=0.0, scalar2=None, op0=Alu.add)
                        else:
                            nc.scalar.copy(dst_sl, pst)
                        ncopy += 1

            # b0 (fp32) and w1 (fp16) as per-partition columns over k-blocks
            b0c = [const.tile([128, 1], f32, name=f"b0c_{k}") for k in range(HB)]
            w1c = [const.tile([128, 1], f16, name=f"w1c_{k}") for k in range(HB)]
            for kb in range(HB):
                k0 = kb * 128
                ksz = min(H, k0 + 128) - k0
                w1f = work.tile([128, 1], f32, tag="w1scratch")
                nc.vector.memset(b0c[kb], 0.0)
                nc.vector.memset(w1c[kb], 0.0)
                nc.sync.dma_start(out=b0c[kb][0:ksz, 0:1], in_=b0_in[k0:k0 + ksz])
                nc.sync.dma_start(out=w1f[0:ksz, 0:1], in_=w1_in[k0:k0 + ksz])
                nc.vector.tensor_scalar(out=w1c[kb][0:ksz, :], in0=w1f[0:ksz, :], scalar1=0.0, scalar2=None, op0=Alu.add)

            # ---------------- phase A: aib = aiT + b0 (fp32), ajT (fp16) -------
            aib = [const.tile([128, N], f32, name=f"aib_{k}") for k in range(HB)]
            ajT = [const.tile([128, N], f16, name=f"ajT_{k}") for k in range(HB)]
            for wT, dst, is_ai in ((w0aT, aib, True), (w0bT, ajT, False)):
                for kb in range(HB):
                    ps = psA.tile([128, N], f32, tag="A")
                    for hb in range(HB):
                        nc.tensor.matmul(
                            ps,
                            lhsT=wT[hb][:, kb * 128:(kb + 1) * 128],
                            rhs=hT[hb],
                            start=(hb == 0),
                            stop=(hb == HB - 1),
                        )
                    if is_ai:
                        nc.vector.tensor_scalar(
                            out=dst[kb], in0=ps, scalar1=b0c[kb], scalar2=None,
                            op0=Alu.add,
                        )
                    else:
                        nc.vector.tensor_scalar(out=dst[kb], in0=ps, scalar1=0.0, scalar2=None, op0=Alu.add)

            # Tail-pair setup: k-block 2 has only 44 real rows, so two queries'
            # tails share one 108-partition op (rows 0:44 = query i, 64:108 =
            # query i+1 via a column-shifted bias layout).
            KT = H - 2 * 128  # 44
            ajT_tail2 = const.tile([128, N], f16)
            aib_tail2 = const.tile([128, N], f32)
            w1c_tail2 = const.tile([128, 1], f16)
            nc.vector.memset(ajT_tail2, 0.0)
            nc.vector.memset(aib_tail2, 0.0)
            nc.vector.memset(w1c_tail2, 0.0)
            nc.vector.tensor_scalar(out=ajT_tail2[0:KT, :], in0=ajT[2][0:KT, :],
                                    scalar1=0.0, scalar2=None, op0=Alu.add)
            nc.vector.tensor_scalar(out=ajT_tail2[64:64 + KT, :], in0=ajT[2][0:KT, :],
                                    scalar1=0.0, scalar2=None, op0=Alu.add)
            nc.vector.tensor_scalar(out=aib_tail2[0:KT, :], in0=aib[2][0:KT, :],
                                    scalar1=0.0, scalar2=None, op0=Alu.add)
            nc.vector.tensor_scalar(out=aib_tail2[64:64 + KT, 0:N - 1],
                                    in0=aib[2][0:KT, 1:N],
                                    scalar1=0.0, scalar2=None, op0=Alu.add)
            nc.vector.tensor_scalar(out=w1c_tail2[0:KT, :], in0=w1c[2][0:KT, :],
                                    scalar1=0.0, scalar2=None, op0=Alu.add)
            nc.vector.tensor_scalar(out=w1c_tail2[64:64 + KT, :], in0=w1c[2][0:KT, :],
                                    scalar1=0.0, scalar2=None, op0=Alu.add)

            # ------- phase B: pT[j, i] columns = w1 . relu(ajT + aib[:, i]) ----
            pT = [psP.tile([128, N], f32, name=f"pT_{jb}") for jb in range(NB)]
            if SKIP_MM:
                nc.vector.memset(pT[1], 0.0)
            opc = 0
            for i0 in range(0, N_I, 2):
                # 4 full-block ops (2 queries x k-blocks 0,1) + 1 shared tail op
                tt = tpool.tile([128, 4 * N], f16, tag="T")
                ttt = tpool.tile([128, N], f16, tag="Tt")
                ops = [(q, kb) for q in range(2) for kb in range(2)] + [(2, 2)]
                pair_sel = ENGINE_PATTERN[(i0 // 2) % len(ENGINE_PATTERN)]
                tail_sel = (TAIL_PATTERN[(i0 // 2) % len(TAIL_PATTERN)]
                            if TAIL_PATTERN else pair_sel)
                for q, kb in ops:
                    if q == 2:
                        out_sl, in_sl = ttt[:, :], ajT_tail2
                        bias = aib_tail2[:, i0:i0 + 1]
                    else:
                        out_sl = tt[:, (q * 2 + kb) * N:(q * 2 + kb + 1) * N]
                        in_sl = ajT[kb]
                        bias = aib[kb][:, i0 + q:i0 + q + 1]
                    sel = tail_sel if q == 2 else pair_sel
                    opc += 1
                    if sel == "A":
                        nc.scalar.activation(out=out_sl, in_=in_sl, func=Relu,
                                             bias=bias, scale=1.0)
                    elif sel == "G":
                        nc.gpsimd.tensor_scalar(out=out_sl, in0=in_sl, scalar1=bias,
                                                scalar2=0.0, op0=Alu.add, op1=Alu.max)
                    else:
                        nc.vector.tensor_scalar(out=out_sl, in0=in_sl, scalar1=bias,
                                                scalar2=0.0, op0=Alu.add, op1=Alu.max)
                for q in range(2):
                    i = i0 + q
                    tb = 64 * q
                    for jb in range(1 if SKIP_MM else NB):
                        for kb in range(2):
                            nc.tensor.matmul(
                                pT[jb][:, i:i + 1],
                                lhsT=tt[:, (q * 2 + kb) * N + jb * 128:
                                        (q * 2 + kb) * N + jb * 128 + 128],
                                rhs=w1c[kb],
                                start=(kb == 0),
                                stop=False,
                            )
                        nc.tensor.matmul(
                            pT[jb][:, i:i + 1],
                            lhsT=ttt[tb:tb + KT, jb * 128:jb * 128 + 128],
                            rhs=w1c_tail2[tb:tb + KT, :],
                            start=False,
                            stop=True,
                        )

            # ---------------- softmax (transposed, no max-subtraction) ---------
            # p is O(1) for this problem (|p| < ~2), so exp never overflows fp16.
            e16 = [const.tile([128, N], f16, name=f"e16_{jb}") for jb in range(NB)]
            for jb in range(NB):
                nc.scalar.activation(out=e16[jb], in_=pT[jb], func=Exp)

            # final: one matmul group per ib gives u = e^T.T @ h AND the row
            # sum s in the appended ones column; y = u * (1/s) per partition
            for ib in range(NB):
                pso = psO.tile([128, H + 1], f32, tag="O")
                for jb in range(NB):
                    nc.tensor.matmul(
                        pso,
                        lhsT=e16[jb][:, ib * 128:(ib + 1) * 128],
                        rhs=h_f16[jb][:, 0:H + 1],
                        start=(jb == 0),
                        stop=(jb == NB - 1),
                    )
                rcol = work.tile([128, 1], f32, tag=f"rcol{ib}")
                nc.vector.reciprocal(rcol, pso[:, H:H + 1])
                yt = work.tile([128, H], f32, tag="y")
                nc.vector.tensor_scalar(
                    out=yt, in0=pso[:, 0:H], scalar1=rcol, scalar2=None, op0=Alu.mult,
                )
                nc.sync.dma_start(out=y_out[ib * 128:(ib + 1) * 128, :], in_=yt)
    return nc


def build_nc():
    nc = bacc.Bacc("TRN2", target_bir_lowering=False, debug=False, num_devices=B)
    _emit(nc)
    nc.compile()
    return nc


def _get_nc():
    if "nc" not in _CACHE:
        _CACHE["nc"] = build_nc()
    return _CACHE["nc"]


def kernel(h_prev, W0, b0, W1, b1, **_ignored):
    del b1  # softmax is invariant to the scalar output bias
    h_prev = np.asarray(h_prev, np.float32)
    W0 = np.asarray(W0, np.float32)
    b0 = np.asarray(b0, np.float32).reshape(H)
    w1 = np.asarray(W1, np.float32).reshape(H)
    assert h_prev.shape == (B, N, H), h_prev.shape

    nc = _get_nc()
    in_maps = [
        {"h": np.ascontiguousarray(h_prev[c]), "w0": W0, "b0": b0, "w1": w1}
        for c in range(B)
    ]
    res = run_bass_kernel_spmd(nc, in_maps, core_ids=list(range(B)))
    return np.stack([res.results[c]["y"] for c in range(B)], axis=0).astype(np.float32)



# revision 2
# speedup vs baseline: 3.1028x; 3.1028x over previous
"""EvidenceLevelAttention (additive attention GNN message passing) on 8 trn2 cores.

Math per batch b (B=8, N=256, H=300):
    ai = h @ W0a.T ; aj = h @ W0b.T                     (W0a = W0[:, :H], W0b = W0[:, H:])
    p[i, j] = w1 . relu(ai[i] + aj[j] + b0)  (+ b1, dropped: softmax shift-invariant)
    a = softmax(p, axis=-1) ;  y = a @ h

Data-parallel: core c computes batch c. Heavy math in fp16 with fp32 PSUM
accumulation.

Pairwise phase: hidden dim k (300 -> padded 384 = 3x128) sits on partitions so
the per-i bias is a per-partition scalar and relu(ajT + aib[:, i]) is ONE fused
DVE tensor_scalar(add, max) per (i, k-block) — all on DVE, which runs this op
shape in its 4x perf mode (~67ns); GpSimd takes ~4us for the same op and the
Act engine ~440ns, so neither touches the hot loop.

The w1 contraction runs the relu tiles through the PE as the MOVING operand:
the stationary is a [128, 128] sliding window of a zero tile with the w1
k-block at one column, so query i's dot products land on psum partition
i mod 128 — p accumulates directly in [i row, j col] layout across a single
320-matmul psum accumulation group per 128 queries. LDWEIGHTS is a 128-col
fp16 load (FWL) fully hidden under the 256-col stream. Tails of a query pair
(k rows 256:300) share one matmul via a two-column stationary window.

Softmax needs no max-subtraction (p is O(1)); exp runs on Act straight out of
PSUM, row sums come from an appended ones column in the final ee^T.T @ h
matmul after PE-transposing exp(p), and 1/s scales the output per partition.
"""

import numpy as np

import concourse.bass as bass
import concourse.mybir as mybir
import concourse.tile as tile
from concourse import bacc
from concourse.bass_utils import run_bass_kernel_spmd
from concourse.masks import make_identity

B, N, H = 8, 256, 300
HB = 3          # hidden-dim blocks of 128
HP = HB * 128   # padded hidden dim
NB = 2          # row blocks of 128
F32 = mybir.dt.float32
F16 = mybir.dt.float16
GROUP = 128     # queries per psum accumulation group
KT = H - 2 * 128  # 44 tail rows of the hidden dim
T_BUFS = 12

_CACHE = {}


def _emit(nc):
    f32, f16 = F32, F16
    Alu = mybir.AluOpType
    Exp = mybir.ActivationFunctionType.Exp

    h_in = nc.dram_tensor("h", [N, H], f32, kind="ExternalInput")
    w0_in = nc.dram_tensor("w0", [H, 2 * H], f32, kind="ExternalInput")
    b0_in = nc.dram_tensor("b0", [H], f32, kind="ExternalInput")
    w1_in = nc.dram_tensor("w1", [H], f32, kind="ExternalInput")
    y_out = nc.dram_tensor("y", [N, H], f32, kind="ExternalOutput")

    with tile.TileContext(nc) as tc:
        with (
            tc.tile_pool(name="const", bufs=1) as const,
            tc.tile_pool(name="work", bufs=2) as work,
            tc.tile_pool(name="tpool", bufs=T_BUFS) as tpool,
            tc.tile_pool(name="psA", bufs=2, space="PSUM") as psA,
            tc.tile_pool(name="psT", bufs=2, space="PSUM") as psT,
            tc.tile_pool(name="psP", bufs=2, space="PSUM") as psP,
            tc.tile_pool(name="psO", bufs=2, space="PSUM") as psO,
        ):
            # ---------------- phase 0: loads, casts, transposes ----------------
            # W0 first: it heads the longest dependency chain (cast->transpose->
            # phase A). k-blocked rows, columns split [W0a | pad | W0b | pad].
            w0_f32 = []
            for kb in range(HB):
                k0 = kb * 128
                ksz = min(H, k0 + 128) - k0
                t32 = work.tile([128, 2 * H], f32, tag=f"w0scratch{kb}")
                nc.sync.dma_start(out=t32[0:ksz, :], in_=w0_in[k0:k0 + ksz, :])
                w0_f32.append((t32, ksz))

            # h rows, fp32 then fp16 (k-padded with zeros)
            h_f32 = [const.tile([128, H], f32, name=f"h_f32_{k}") for k in range(NB)]
            h_f16 = [const.tile([128, HP], f16, name=f"h_f16_{k}") for k in range(NB)]
            for ib in range(NB):
                nc.sync.dma_start(out=h_f32[ib], in_=h_in[ib * 128:(ib + 1) * 128, :])
                nc.vector.memset(h_f16[ib][:, H:HP], 0.0)
                nc.vector.memset(h_f16[ib][:, H:H + 1], 1.0)  # ones col for fused row-sum
                nc.vector.tensor_scalar(out=h_f16[ib][:, 0:H], in0=h_f32[ib], scalar1=0.0, scalar2=None, op0=Alu.add)

            w0_f16 = []
            for kb in range(HB):
                t32, ksz = w0_f32[kb]
                tf = const.tile([128, 2 * HP], f16, name=f"w0f16_{kb}")
                nc.vector.memset(tf, 0.0)
                nc.vector.tensor_scalar(out=tf[0:ksz, 0:H], in0=t32[0:ksz, 0:H], scalar1=0.0, scalar2=None, op0=Alu.add)
                nc.vector.tensor_scalar(out=tf[0:ksz, HP:HP + H], in0=t32[0:ksz, H:2 * H], scalar1=0.0, scalar2=None, op0=Alu.add)
                w0_f16.append(tf)

            # hT[hb]: [128 h, 256 n]  (PE transpose of fp16 tiles)
            ident = const.tile([128, 128], f16)
            make_identity(nc, ident)
            hT = [const.tile([128, N], f16, name=f"hT_{k}") for k in range(HB)]
            ncopy = 0
            for hb in range(HB):
                for ib in range(NB):
                    pst = psT.tile([128, 128], f16, tag="tr")
                    nc.tensor.transpose(
                        pst, h_f16[ib][:, hb * 128:(hb + 1) * 128], ident,
                    )
                    dst_sl = hT[hb][:, ib * 128:(ib + 1) * 128]
                    if ncopy % 2 == 0:
                        nc.vector.tensor_scalar(out=dst_sl, in0=pst, scalar1=0.0, scalar2=None, op0=Alu.add)
                    else:
                        nc.scalar.copy(dst_sl, pst)
                    ncopy += 1

            # W0aT/W0bT[hb]: [128 h, 384 k] via PE transpose (128x128 blocks)
            w0aT = [const.tile([128, HP], f16, name=f"w0aT_{k}") for k in range(HB)]
            w0bT = [const.tile([128, HP], f16, name=f"w0bT_{k}") for k in range(HB)]
            for half, dst in ((0, w0aT), (1, w0bT)):
                for hb in range(HB):
                    for kb in range(HB):
                        pst = psT.tile([128, 128], f16, tag="tr")
                        nc.tensor.transpose(
                            pst,
                            w0_f16[kb][:, half * HP + hb * 128: half * HP + (hb + 1) * 128],
                            ident,
                        )
                        dst_sl = dst[hb][:, kb * 128:(kb + 1) * 128]
                        if ncopy % 2 == 0:
                            nc.vector.tensor_scalar(out=dst_sl, in0=pst, scalar1=0.0, scalar2=None, op0=Alu.add)
                        else:
                            nc.scalar.copy(dst_sl, pst)
                        ncopy += 1

            # b0 (fp32) and w1 (fp16) as per-partition columns over k-blocks
            b0c = [const.tile([128, 1], f32, name=f"b0c_{k}") for k in range(HB)]
            w1c = [const.tile([128, 1], f16, name=f"w1c_{k}") for k in range(HB)]
            for kb in range(HB):
                k0 = kb * 128
                ksz = min(H, k0 + 128) - k0
                w1f = work.tile([128, 1], f32, tag="w1scratch")
                nc.vector.memset(b0c[kb], 0.0)
                nc.vector.memset(w1c[kb], 0.0)
                nc.sync.dma_start(out=b0c[kb][0:ksz, 0:1], in_=b0_in[k0:k0 + ksz])
                nc.sync.dma_start(out=w1f[0:ksz, 0:1], in_=w1_in[k0:k0 + ksz])
                nc.vector.tensor_scalar(out=w1c[kb][0:ksz, :], in0=w1f[0:ksz, :], scalar1=0.0, scalar2=None, op0=Alu.add)

            # Sliding-window stationaries for the w1 contraction: Z[kb] is zero
            # except col 127 = w1 k-block; lhsT = Z[kb][:, 127-m : 255-m] puts
            # w1 at stationary column m, so query i=m's dot products land on
            # psum partition m. ZT packs both tail halves of a query pair:
            # col 127 rows 0:44 and col 128 rows 64:108 (matching the packed
            # tail tile layout below), so one matmul does both queries' tails.
            Z = [const.tile([128, 255], f16, name=f"Z_{kb}") for kb in range(2)]
            for kb in range(2):
                nc.vector.memset(Z[kb], 0.0)
                nc.vector.tensor_scalar(out=Z[kb][:, 127:128], in0=w1c[kb], scalar1=0.0, scalar2=None, op0=Alu.add)
            ZT = const.tile([128, 255], f16, name="ZT")
            nc.vector.memset(ZT, 0.0)
            nc.vector.tensor_scalar(out=ZT[0:KT, 127:128], in0=w1c[2][0:KT, :], scalar1=0.0, scalar2=None, op0=Alu.add)
            nc.vector.tensor_scalar(out=ZT[64:64 + KT, 128:129], in0=w1c[2][0:KT, :], scalar1=0.0, scalar2=None, op0=Alu.add)

            # ---------------- phase A: aib = aiT + b0 (fp32), ajT (fp16) -------
            aib = [const.tile([128, N], f32, name=f"aib_{k}") for k in range(HB)]
            ajT = [const.tile([128, N], f16, name=f"ajT_{k}") for k in range(HB)]
            for wT, dst, is_ai in ((w0aT, aib, True), (w0bT, ajT, False)):
                for kb in range(HB):
                    ps = psA.tile([128, N], f32, tag="A")
                    for hb in range(HB):
                        nc.tensor.matmul(
                            ps,
                            lhsT=wT[hb][:, kb * 128:(kb + 1) * 128],
                            rhs=hT[hb],
                            start=(hb == 0),
                            stop=(hb == HB - 1),
                        )
                    if is_ai:
                        nc.vector.tensor_scalar(
                            out=dst[kb], in0=ps, scalar1=b0c[kb], scalar2=None,
                            op0=Alu.add,
                        )
                    else:
                        nc.vector.tensor_scalar(out=dst[kb], in0=ps, scalar1=0.0, scalar2=None, op0=Alu.add)

            # Tail-pair setup: k-block 2 has only 44 real rows, so two queries'
            # tails share one op/matmul (rows 0:44 = query i, 64:108 = query
            # i+1 via a column-shifted bias layout).
            ajT_tail2 = const.tile([128, N], f16)
            aib_tail2 = const.tile([128, N], f32)
            nc.vector.memset(ajT_tail2, 0.0)
            nc.vector.memset(aib_tail2, 0.0)
            nc.vector.tensor_scalar(out=ajT_tail2[0:KT, :], in0=ajT[2][0:KT, :],
                                    scalar1=0.0, scalar2=None, op0=Alu.add)
            nc.vector.tensor_scalar(out=ajT_tail2[64:64 + KT, :], in0=ajT[2][0:KT, :],
                                    scalar1=0.0, scalar2=None, op0=Alu.add)
            nc.vector.tensor_scalar(out=aib_tail2[0:KT, :], in0=aib[2][0:KT, :],
                                    scalar1=0.0, scalar2=None, op0=Alu.add)
            nc.vector.tensor_scalar(out=aib_tail2[64:64 + KT, 0:N - 1],
                                    in0=aib[2][0:KT, 1:N],
                                    scalar1=0.0, scalar2=None, op0=Alu.add)

            # ------- phase B: p[i, j] = w1 . relu(ajT[:, j] + aib[:, i]) -------
            # eIJ[ib][i % 128, j] = exp(p[i, j]) for i in block ib
            eIJ = [const.tile([128, N], f16, name=f"eIJ_{ib}") for ib in range(NB)]
            Pg = None
            for i0 in range(0, N, 2):
                g, m0 = divmod(i0, GROUP)
                if m0 == 0:
                    Pg = psP.tile([128, N], f32, tag="P")
                tt = tpool.tile([128, 4 * N], f16, tag="T")
                ttt = tpool.tile([128, N], f16, tag="Tt")
                for q in range(2):
                    for kb in range(2):
                        nc.vector.tensor_scalar(
                            out=tt[:, (q * 2 + kb) * N:(q * 2 + kb + 1) * N],
                            in0=ajT[kb],
                            scalar1=aib[kb][:, i0 + q:i0 + q + 1], scalar2=0.0,
                            op0=Alu.add, op1=Alu.max)
                nc.vector.tensor_scalar(
                    out=ttt, in0=ajT_tail2,
                    scalar1=aib_tail2[:, i0:i0 + 1], scalar2=0.0,
                    op0=Alu.add, op1=Alu.max)
                first = (m0 == 0)
                for q in range(2):
                    m = m0 + q
                    for kb in range(2):
                        nc.tensor.matmul(
                            Pg,
                            lhsT=Z[kb][:, 127 - m:255 - m],
                            rhs=tt[:, (q * 2 + kb) * N:(q * 2 + kb + 1) * N],
                            start=first,
                            stop=False,
                        )
                        first = False
                nc.tensor.matmul(
                    Pg,
                    lhsT=ZT[:, 127 - m0:255 - m0],
                    rhs=ttt,
                    start=False,
                    stop=(m0 == GROUP - 2),
                )
                if m0 == GROUP - 2:
                    nc.scalar.activation(out=eIJ[g], in_=Pg, func=Exp)

            # ---------------- softmax + output --------------------------------
            # eT[jb]: [128 j, 256 i] — PE transpose of exp(p) blocks
            eT = [const.tile([128, N], f16, name=f"eT_{jb}") for jb in range(NB)]
            for ib in range(NB):
                for jb in range(NB):
                    pst = psT.tile([128, 128], f16, tag="tr")
                    nc.tensor.transpose(pst, eIJ[ib][:, jb * 128:(jb + 1) * 128], ident)
                    if (ib + jb) % 2 == 0:
                        nc.vector.tensor_scalar(out=eT[jb][:, ib * 128:(ib + 1) * 128], in0=pst, scalar1=0.0, scalar2=None, op0=Alu.add)
                    else:
                        nc.scalar.copy(eT[jb][:, ib * 128:(ib + 1) * 128], pst)

            # final: one matmul group per ib gives u = e^T.T @ h AND the row
            # sum s in the appended ones column; y = u * (1/s) per partition
            for ib in range(NB):
                pso = psO.tile([128, H + 1], f32, tag="O")
                for jb in range(NB):
                    nc.tensor.matmul(
                        pso,
                        lhsT=eT[jb][:, ib * 128:(ib + 1) * 128],
                        rhs=h_f16[jb][:, 0:H + 1],
                        start=(jb == 0),
                        stop=(jb == NB - 1),
                    )
                rcol = work.tile([128, 1], f32, tag=f"rcol{ib}")
                nc.vector.reciprocal(rcol, pso[:, H:H + 1])
                yt = work.tile([128, H], f32, tag="y")
                nc.vector.tensor_scalar(
                    out=yt, in0=pso[:, 0:H], scalar1=rcol, scalar2=None, op0=Alu.mult,
                )
                nc.sync.dma_start(out=y_out[ib * 128:(ib + 1) * 128, :], in_=yt)
    return nc


def build_nc():
    nc = bacc.Bacc("TRN2", target_bir_lowering=False, debug=False, num_devices=B)
    _emit(nc)
    nc.compile()
    return nc


def _get_nc():
    if "nc" not in _CACHE:
        _CACHE["nc"] = build_nc()
    return _CACHE["nc"]


def kernel(h_prev, W0, b0, W1, b1, **_ignored):
    del b1  # softmax is invariant to the scalar output bias
    h_prev = np.asarray(h_prev, np.float32)
    W0 = np.asarray(W0, np.float32)
    b0 = np.asarray(b0, np.float32).reshape(H)
    w1 = np.asarray(W1, np.float32).reshape(H)
    assert h_prev.shape == (B, N, H), h_prev.shape

    nc = _get_nc()
    in_maps = [
        {"h": np.ascontiguousarray(h_prev[c]), "w0": W0, "b0": b0, "w1": w1}
        for c in range(B)
    ]
    res = run_bass_kernel_spmd(nc, in_maps, core_ids=list(range(B)))
    return np.stack([res.results[c]["y"] for c in range(B)], axis=0).astype(np.float32)


# revision 5
# speedup vs baseline: 3.9713x; 1.2799x over previous
"""EvidenceLevelAttention (additive attention GNN message passing) on 8 trn2 cores.

Math per batch b (B=8, N=256, H=300):
    ai = h @ W0a.T ; aj = h @ W0b.T                     (W0a = W0[:, :H], W0b = W0[:, H:])
    p[i, j] = w1 . relu(ai[i] + aj[j] + b0)  (+ b1, dropped: softmax shift-invariant)
    a = softmax(p, axis=-1) ;  y = a @ h

Data-parallel: core c computes batch c. Heavy math in fp16 with fp32 PSUM
accumulation.

Pairwise phase: hidden dim k (300 -> padded 384 = 3x128) sits on partitions so
the per-i bias is a per-partition scalar and relu(ajT + aib[:, i]) is ONE fused
DVE tensor_scalar(add, max) per (i, k-block) — all on DVE, which runs this op
shape in its 4x perf mode (~67ns); GpSimd takes ~4us for the same op and the
Act engine ~440ns, so neither touches the hot loop.

The w1 contraction runs the relu tiles through the PE as the MOVING operand:
the stationary is a [128, 128] sliding window of a zero tile with the w1
k-block at one column, so query i's dot products land on psum partition
i mod 128 — p accumulates directly in [i row, j col] layout across a single
320-matmul psum accumulation group per 128 queries. LDWEIGHTS is a 128-col
fp16 load (FWL) fully hidden under the 256-col stream. Tails of a query pair
(k rows 256:300) share one matmul via a two-column stationary window.

Softmax needs no max-subtraction (p is O(1)); exp runs on Act straight out of
PSUM, row sums come from an appended ones column in the final ee^T.T @ h
matmul after PE-transposing exp(p), and 1/s scales the output per partition.
"""

import numpy as np

import concourse.bass as bass
import concourse.mybir as mybir
import concourse.tile as tile
from concourse import bacc
from concourse.bass_utils import run_bass_kernel_spmd
from concourse.masks import make_identity

B, N, H = 8, 256, 300
HB = 3          # hidden-dim blocks of 128
HP = HB * 128   # padded hidden dim
NB = 2          # row blocks of 128
F32 = mybir.dt.float32
F16 = mybir.dt.float16
GROUP = 128     # queries per psum accumulation group
KT = H - 2 * 128  # 44 tail rows of the hidden dim
T_BUFS = 12

_CACHE = {}


def _emit(nc):
    f32, f16 = F32, F16
    Alu = mybir.AluOpType
    Exp = mybir.ActivationFunctionType.Exp

    h_in = nc.dram_tensor("h", [N, H], f32, kind="ExternalInput")
    w0_in = nc.dram_tensor("w0", [H, 2 * H], f32, kind="ExternalInput")
    b0_in = nc.dram_tensor("b0", [H], f32, kind="ExternalInput")
    w1_in = nc.dram_tensor("w1", [H], f32, kind="ExternalInput")
    y_out = nc.dram_tensor("y", [N, H], f32, kind="ExternalOutput")

    with tile.TileContext(nc) as tc:
        with (
            tc.tile_pool(name="const", bufs=1) as const,
            tc.tile_pool(name="work", bufs=2) as work,
            tc.tile_pool(name="tpool", bufs=T_BUFS) as tpool,
            tc.tile_pool(name="psA", bufs=2, space="PSUM") as psA,
            tc.tile_pool(name="psT", bufs=2, space="PSUM") as psT,
            tc.tile_pool(name="psP", bufs=2, space="PSUM") as psP,
            tc.tile_pool(name="psO", bufs=2, space="PSUM") as psO,
        ):
            # ---------------- phase 0: loads, casts, transposes ----------------
            # W0 first: it heads the longest dependency chain (cast->transpose->
            # phase A). k-blocked rows, columns split [W0a | pad | W0b | pad].
            w0_f32 = []
            for kb in range(HB):
                k0 = kb * 128
                ksz = min(H, k0 + 128) - k0
                t32 = work.tile([128, 2 * H], f32, tag=f"w0scratch{kb}")
                nc.sync.dma_start(out=t32[0:ksz, :], in_=w0_in[k0:k0 + ksz, :])
                w0_f32.append((t32, ksz))

            # h rows, fp32 then fp16 (k-padded with zeros)
            h_f32 = [const.tile([128, H], f32, name=f"h_f32_{k}") for k in range(NB)]
            h_f16 = [const.tile([128, HP], f16, name=f"h_f16_{k}") for k in range(NB)]
            for ib in range(NB):
                nc.sync.dma_start(out=h_f32[ib], in_=h_in[ib * 128:(ib + 1) * 128, :])
                nc.vector.memset(h_f16[ib][:, H:HP], 0.0)
                nc.vector.memset(h_f16[ib][:, H:H + 1], 1.0)  # ones col for fused row-sum
                nc.vector.tensor_scalar(out=h_f16[ib][:, 0:H], in0=h_f32[ib], scalar1=0.0, scalar2=None, op0=Alu.add)

            w0_f16 = []
            for kb in range(HB):
                t32, ksz = w0_f32[kb]
                tf = const.tile([128, 2 * HP], f16, name=f"w0f16_{kb}")
                nc.vector.memset(tf, 0.0)
                nc.vector.tensor_scalar(out=tf[0:ksz, 0:H], in0=t32[0:ksz, 0:H], scalar1=0.0, scalar2=None, op0=Alu.add)
                nc.vector.tensor_scalar(out=tf[0:ksz, HP:HP + H], in0=t32[0:ksz, H:2 * H], scalar1=0.0, scalar2=None, op0=Alu.add)
                w0_f16.append(tf)

            # hT[hb]: [128 h, 256 n]  (PE transpose of fp16 tiles)
            ident = const.tile([128, 128], f16)
            make_identity(nc, ident)
            hT = [const.tile([128, N], f16, name=f"hT_{k}") for k in range(HB)]
            ncopy = 0
            for hb in range(HB):
                for ib in range(NB):
                    pst = psT.tile([128, 128], f16, tag="tr")
                    nc.tensor.transpose(
                        pst, h_f16[ib][:, hb * 128:(hb + 1) * 128], ident,
                    )
                    dst_sl = hT[hb][:, ib * 128:(ib + 1) * 128]
                    if ncopy % 2 == 0:
                        nc.vector.tensor_scalar(out=dst_sl, in0=pst, scalar1=0.0, scalar2=None, op0=Alu.add)
                    else:
                        nc.scalar.copy(dst_sl, pst)
                    ncopy += 1

            # W0aT/W0bT[hb]: [128 h, 384 k] via PE transpose (128x128 blocks)
            w0aT = [const.tile([128, HP], f16, name=f"w0aT_{k}") for k in range(HB)]
            w0bT = [const.tile([128, HP], f16, name=f"w0bT_{k}") for k in range(HB)]
            for half, dst in ((0, w0aT), (1, w0bT)):
                for hb in range(HB):
                    for kb in range(HB):
                        pst = psT.tile([128, 128], f16, tag="tr")
                        nc.tensor.transpose(
                            pst,
                            w0_f16[kb][:, half * HP + hb * 128: half * HP + (hb + 1) * 128],
                            ident,
                        )
                        dst_sl = dst[hb][:, kb * 128:(kb + 1) * 128]
                        if ncopy % 2 == 0:
                            nc.vector.tensor_scalar(out=dst_sl, in0=pst, scalar1=0.0, scalar2=None, op0=Alu.add)
                        else:
                            nc.scalar.copy(dst_sl, pst)
                        ncopy += 1

            # b0 (fp32) and w1 (fp16) as per-partition columns over k-blocks
            b0c = [const.tile([128, 1], f32, name=f"b0c_{k}") for k in range(HB)]
            w1c = [const.tile([128, 1], f16, name=f"w1c_{k}") for k in range(HB)]
            for kb in range(HB):
                k0 = kb * 128
                ksz = min(H, k0 + 128) - k0
                w1f = work.tile([128, 1], f32, tag="w1scratch")
                nc.vector.memset(b0c[kb], 0.0)
                nc.vector.memset(w1c[kb], 0.0)
                nc.sync.dma_start(out=b0c[kb][0:ksz, 0:1], in_=b0_in[k0:k0 + ksz])
                nc.sync.dma_start(out=w1f[0:ksz, 0:1], in_=w1_in[k0:k0 + ksz])
                nc.vector.tensor_scalar(out=w1c[kb][0:ksz, :], in0=w1f[0:ksz, :], scalar1=0.0, scalar2=None, op0=Alu.add)

            # Sliding-window stationaries for the w1 contraction: Z[kb] is zero
            # except col 31 = w1 k-block; lhsT = Z[kb][:, 31-mm : 63-mm] puts
            # w1 at stationary column mm, so query i's dot products land on
            # psum partition 32*(i//32 % 4) + mm (the matmul writes a 32-row
            # PSUM slab selected by the out AP's base partition). The 32-col
            # stationary keeps LDWEIGHTS at ~27ns vs ~97ns for a 128-col load.
            # ZT packs both tail halves of a query pair: col 31 rows 0:44 and
            # col 32 rows 64:108 (matching the packed tail tile layout below),
            # so one matmul does both queries' tails.
            Z = [const.tile([128, 63], f16, name=f"Z_{kb}") for kb in range(2)]
            for kb in range(2):
                nc.vector.memset(Z[kb], 0.0)
                nc.vector.tensor_scalar(out=Z[kb][:, 31:32], in0=w1c[kb], scalar1=0.0, scalar2=None, op0=Alu.add)
            ZT = const.tile([128, 63], f16, name="ZT")
            nc.vector.memset(ZT, 0.0)
            nc.vector.tensor_scalar(out=ZT[0:KT, 31:32], in0=w1c[2][0:KT, :], scalar1=0.0, scalar2=None, op0=Alu.add)
            nc.vector.tensor_scalar(out=ZT[64:64 + KT, 32:33], in0=w1c[2][0:KT, :], scalar1=0.0, scalar2=None, op0=Alu.add)

            # ---------------- phase A: aib = aiT + b0 (fp32), ajT (fp16) -------
            aib = [const.tile([128, N], f32, name=f"aib_{k}") for k in range(HB)]
            ajT = [const.tile([128, N], f16, name=f"ajT_{k}") for k in range(HB)]
            for wT, dst, is_ai in ((w0aT, aib, True), (w0bT, ajT, False)):
                for kb in range(HB):
                    ps = psA.tile([128, N], f32, tag="A")
                    for hb in range(HB):
                        nc.tensor.matmul(
                            ps,
                            lhsT=wT[hb][:, kb * 128:(kb + 1) * 128],
                            rhs=hT[hb],
                            start=(hb == 0),
                            stop=(hb == HB - 1),
                        )
                    if is_ai:
                        nc.vector.tensor_scalar(
                            out=dst[kb], in0=ps, scalar1=b0c[kb], scalar2=None,
                            op0=Alu.add,
                        )
                    else:
                        nc.vector.tensor_scalar(out=dst[kb], in0=ps, scalar1=0.0, scalar2=None, op0=Alu.add)

            # Tail-pair setup: k-block 2 has only 44 real rows, so two queries'
            # tails share one op/matmul (rows 0:44 = query i, 64:108 = query
            # i+1 via a column-shifted bias layout).
            ajT_tail2 = const.tile([128, N], f16)
            aib_tail2 = const.tile([128, N], f32)
            nc.vector.memset(ajT_tail2, 0.0)
            nc.vector.memset(aib_tail2, 0.0)
            nc.vector.tensor_scalar(out=ajT_tail2[0:KT, :], in0=ajT[2][0:KT, :],
                                    scalar1=0.0, scalar2=None, op0=Alu.add)
            nc.vector.tensor_scalar(out=ajT_tail2[64:64 + KT, :], in0=ajT[2][0:KT, :],
                                    scalar1=0.0, scalar2=None, op0=Alu.add)
            nc.vector.tensor_scalar(out=aib_tail2[0:KT, :], in0=aib[2][0:KT, :],
                                    scalar1=0.0, scalar2=None, op0=Alu.add)
            nc.vector.tensor_scalar(out=aib_tail2[64:64 + KT, 0:N - 1],
                                    in0=aib[2][0:KT, 1:N],
                                    scalar1=0.0, scalar2=None, op0=Alu.add)

            # ------- phase B: p[i, j] = w1 . relu(ajT[:, j] + aib[:, i]) -------
            # eIJ[ib][i % 128, j] = exp(p[i, j]) for i in block ib. Elementwise
            # relu runs on DVE (~196ns/op) with ~1.5 ops/pair offloaded to the
            # Act engine (~443ns/op) so both engines finish together; GpSimd
            # (~4us/op for this shape) is never used.
            Relu = mybir.ActivationFunctionType.Relu
            eIJ = [const.tile([128, N], f16, name=f"eIJ_{ib}") for ib in range(NB)]
            Pg = None
            for i0 in range(0, N, 2):
                g, m0 = divmod(i0, GROUP)
                slab, mm = divmod(m0, 32)
                if m0 == 0:
                    Pg = psP.tile([128, N], f32, tag="P")
                odd_pair = (i0 // 2) % 2 == 1
                tt = tpool.tile([128, 4 * N], f16, tag="T")
                ttt = tpool.tile([128, N], f16, tag="Tt")
                for q in range(2):
                    for kb in range(2):
                        out_sl = tt[:, (q * 2 + kb) * N:(q * 2 + kb + 1) * N]
                        bias = aib[kb][:, i0 + q:i0 + q + 1]
                        if odd_pair and q == 1 and kb == 1:
                            nc.scalar.activation(out=out_sl, in_=ajT[kb], func=Relu,
                                                 bias=bias, scale=1.0)
                        else:
                            nc.vector.tensor_scalar(
                                out=out_sl, in0=ajT[kb],
                                scalar1=bias, scalar2=0.0,
                                op0=Alu.add, op1=Alu.max)
                nc.scalar.activation(out=ttt, in_=ajT_tail2, func=Relu,
                                     bias=aib_tail2[:, i0:i0 + 1], scale=1.0)
                out_sl = Pg[32 * slab:32 * (slab + 1), :]
                tpos = (0, 32 * slab)
                first = (mm == 0)
                for q in range(2):
                    m = mm + q
                    for kb in range(2):
                        nc.tensor.matmul(
                            out_sl,
                            lhsT=Z[kb][:, 31 - m:63 - m],
                            rhs=tt[:, (q * 2 + kb) * N:(q * 2 + kb + 1) * N],
                            start=first,
                            stop=False,
                            tile_position=tpos,
                        )
                        first = False
                nc.tensor.matmul(
                    out_sl,
                    lhsT=ZT[:, 31 - mm:63 - mm],
                    rhs=ttt,
                    start=False,
                    stop=(mm == 30),
                    tile_position=tpos,
                )
                if m0 == GROUP - 2:
                    nc.scalar.activation(out=eIJ[g], in_=Pg, func=Exp)

            # ---------------- softmax + output --------------------------------
            # eT[jb]: [128 j, 256 i] — PE transpose of exp(p) blocks
            eT = [const.tile([128, N], f16, name=f"eT_{jb}") for jb in range(NB)]
            for ib in range(NB):
                for jb in range(NB):
                    pst = psT.tile([128, 128], f16, tag="tr")
                    nc.tensor.transpose(pst, eIJ[ib][:, jb * 128:(jb + 1) * 128], ident)
                    if (ib + jb) % 2 == 0:
                        nc.vector.tensor_scalar(out=eT[jb][:, ib * 128:(ib + 1) * 128], in0=pst, scalar1=0.0, scalar2=None, op0=Alu.add)
                    else:
                        nc.scalar.copy(eT[jb][:, ib * 128:(ib + 1) * 128], pst)

            # final: one matmul group per ib gives u = e^T.T @ h AND the row
            # sum s in the appended ones column; y = u * (1/s) per partition
            for ib in range(NB):
                pso = psO.tile([128, H + 1], f32, tag="O")
                for jb in range(NB):
                    nc.tensor.matmul(
                        pso,
                        lhsT=eT[jb][:, ib * 128:(ib + 1) * 128],
                        rhs=h_f16[jb][:, 0:H + 1],
                        start=(jb == 0),
                        stop=(jb == NB - 1),
                    )
                rcol = work.tile([128, 1], f32, tag=f"rcol{ib}")
                nc.vector.reciprocal(rcol, pso[:, H:H + 1])
                yt = work.tile([128, H], f32, tag="y")
                nc.vector.tensor_scalar(
                    out=yt, in0=pso[:, 0:H], scalar1=rcol, scalar2=None, op0=Alu.mult,
                )
                nc.sync.dma_start(out=y_out[ib * 128:(ib + 1) * 128, :], in_=yt)
    return nc


def build_nc():
    nc = bacc.Bacc("TRN2", target_bir_lowering=False, debug=False, num_devices=B)
    _emit(nc)
    nc.compile()
    return nc


def _get_nc():
    if "nc" not in _CACHE:
        _CACHE["nc"] = build_nc()
    return _CACHE["nc"]


def kernel(h_prev, W0, b0, W1, b1, **_ignored):
    del b1  # softmax is invariant to the scalar output bias
    h_prev = np.asarray(h_prev, np.float32)
    W0 = np.asarray(W0, np.float32)
    b0 = np.asarray(b0, np.float32).reshape(H)
    w1 = np.asarray(W1, np.float32).reshape(H)
    assert h_prev.shape == (B, N, H), h_prev.shape

    nc = _get_nc()
    in_maps = [
        {"h": np.ascontiguousarray(h_prev[c]), "w0": W0, "b0": b0, "w1": w1}
        for c in range(B)
    ]
    res = run_bass_kernel_spmd(nc, in_maps, core_ids=list(range(B)))
    return np.stack([res.results[c]["y"] for c in range(B)], axis=0).astype(np.float32)


# revision 10
# speedup vs baseline: 4.0533x; 1.0207x over previous
"""EvidenceLevelAttention (additive attention GNN message passing) on 8 trn2 cores.

Math per batch b (B=8, N=256, H=300):
    ai = h @ W0a.T ; aj = h @ W0b.T                     (W0a = W0[:, :H], W0b = W0[:, H:])
    p[i, j] = w1 . relu(ai[i] + aj[j] + b0)  (+ b1, dropped: softmax shift-invariant)
    a = softmax(p, axis=-1) ;  y = a @ h

Data-parallel: core c computes batch c. Heavy math in fp16 with fp32 PSUM
accumulation.

Pairwise phase: hidden dim k (300 -> padded 384 = 3x128) sits on partitions so
the per-i bias is a per-partition scalar and relu(ajT + aib[:, i]) is ONE fused
tensor_scalar(add, max) / activation(Relu, bias) per (i, k-block). The ops are
split ~2:1 between DVE (~197ns/op) and the Act engine (~400ns/op) so both
engines drain together; GpSimd takes ~4us for this op shape and only gets
memsets. DMA XBAR transposes are avoided too: each dma_start_transpose costs
>1us of issue time on its queue.

The w1 contraction runs the relu tiles through the PE as the MOVING operand:
the stationary is a [128, 32] sliding window of a zero tile with the w1
k-block at one column, so query i's dot products land on psum partition
i mod 128 (32-row slab selected via the out AP / tile_position, column within
the slab by the window shift). p accumulates directly in [i row, j col] layout,
one 32-query slab at a time, inside two [128, 256] psum group tiles. The 32-col
stationary keeps LDWEIGHTS short enough to hide under the 256-col stream
(~128ns/matmul). Tails of a query pair (k rows 256:300) share one matmul via a
two-column stationary window.

Softmax needs no max-subtraction (p is O(1)); exp runs on Act straight out of
PSUM, row sums come from an appended ones column in the final eT.T @ h matmul,
and 1/s scales the output per partition.
"""

import numpy as np

import concourse.bass as bass
import concourse.mybir as mybir
import concourse.tile as tile
from concourse import bacc
from concourse.bass_utils import run_bass_kernel_spmd
from concourse.masks import make_identity

B, N, H = 8, 256, 300
HB = 3          # hidden-dim blocks of 128
HP = HB * 128   # padded hidden dim
NB = 2          # row blocks of 128
F32 = mybir.dt.float32
F16 = mybir.dt.float16
GROUP = 128     # queries per psum accumulation group
KT = H - 2 * 128  # 44 tail rows of the hidden dim
T_BUFS = 12

_CACHE = {}


def _emit(nc):
    f32, f16 = F32, F16
    Alu = mybir.AluOpType
    Exp = mybir.ActivationFunctionType.Exp
    Relu = mybir.ActivationFunctionType.Relu
    Copy = mybir.ActivationFunctionType.Copy

    h_in = nc.dram_tensor("h", [N, H], f32, kind="ExternalInput")
    w0_in = nc.dram_tensor("w0", [H, 2 * H], f32, kind="ExternalInput")
    b0_in = nc.dram_tensor("b0", [H], f32, kind="ExternalInput")
    w1_in = nc.dram_tensor("w1", [H], f32, kind="ExternalInput")
    y_out = nc.dram_tensor("y", [N, H], f32, kind="ExternalOutput")

    with tile.TileContext(nc) as tc:
        with (
            tc.tile_pool(name="const", bufs=1) as const,
            tc.tile_pool(name="work", bufs=2) as work,
            tc.tile_pool(name="tpool", bufs=T_BUFS) as tpool,
            tc.tile_pool(name="psA", bufs=2, space="PSUM") as psA,
            tc.tile_pool(name="psT", bufs=2, space="PSUM") as psT,
            tc.tile_pool(name="psP", bufs=2, space="PSUM") as psP,
            tc.tile_pool(name="psO", bufs=2, space="PSUM") as psO,
        ):
            # ---------------- phase 0: loads, casts, transposes ----------------
            # W0 first: it heads the longest dependency chain (cast ->
            # transpose -> phase A). k-blocked rows, cols [W0a | pad | W0b | pad].
            w0_f32 = []
            for kb in range(HB):
                k0 = kb * 128
                ksz = min(H, k0 + 128) - k0
                t32 = work.tile([128, 2 * H], f32, tag=f"w0scratch{kb}")
                nc.sync.dma_start(out=t32[0:ksz, :], in_=w0_in[k0:k0 + ksz, :])
                w0_f32.append((t32, ksz))

            # h rows, fp32 then fp16 (k-padded with zeros)
            h_f32 = [const.tile([128, H], f32, name=f"h_f32_{k}") for k in range(NB)]
            h_f16 = [const.tile([128, HP], f16, name=f"h_f16_{k}") for k in range(NB)]
            for ib in range(NB):
                nc.sync.dma_start(out=h_f32[ib], in_=h_in[ib * 128:(ib + 1) * 128, :])
                nc.gpsimd.memset(h_f16[ib][:, H:HP], 0.0)
                nc.gpsimd.memset(h_f16[ib][:, H:H + 1], 1.0)  # ones col for fused row-sum
                nc.vector.tensor_scalar(out=h_f16[ib][:, 0:H], in0=h_f32[ib], scalar1=0.0, scalar2=None, op0=Alu.add)

            # b0 (fp32) and w1 (fp16) as per-partition columns over k-blocks
            b0c = [const.tile([128, 1], f32, name=f"b0c_{k}") for k in range(HB)]
            w1c = [const.tile([128, 1], f16, name=f"w1c_{k}") for k in range(HB)]
            for kb in range(HB):
                k0 = kb * 128
                ksz = min(H, k0 + 128) - k0
                w1f = work.tile([128, 1], f32, tag="w1scratch")
                nc.gpsimd.memset(b0c[kb], 0.0)
                nc.gpsimd.memset(w1c[kb], 0.0)
                nc.sync.dma_start(out=b0c[kb][0:ksz, 0:1], in_=b0_in[k0:k0 + ksz])
                nc.sync.dma_start(out=w1f[0:ksz, 0:1], in_=w1_in[k0:k0 + ksz])
                nc.vector.tensor_scalar(out=w1c[kb][0:ksz, :], in0=w1f[0:ksz, :], scalar1=0.0, scalar2=None, op0=Alu.add)

            # Sliding-window stationaries for the w1 contraction: Z[kb] is zero
            # except col 31 = w1 k-block; lhsT = Z[kb][:, 31-mm : 63-mm] puts
            # w1 at stationary column mm. ZT packs both tail halves of a query
            # pair: col 31 rows 0:44 and col 32 rows 64:108 (matching the
            # packed tail tile layout), so one matmul does both queries' tails.
            Z = [const.tile([128, 63], f16, name=f"Z_{kb}") for kb in range(2)]
            for kb in range(2):
                nc.gpsimd.memset(Z[kb], 0.0)
                nc.vector.tensor_scalar(out=Z[kb][:, 31:32], in0=w1c[kb], scalar1=0.0, scalar2=None, op0=Alu.add)
            ZT = const.tile([128, 63], f16, name="ZT")
            nc.gpsimd.memset(ZT, 0.0)
            nc.vector.tensor_scalar(out=ZT[0:KT, 31:32], in0=w1c[2][0:KT, :], scalar1=0.0, scalar2=None, op0=Alu.add)
            nc.vector.tensor_scalar(out=ZT[64:64 + KT, 32:33], in0=w1c[2][0:KT, :], scalar1=0.0, scalar2=None, op0=Alu.add)

            # fp16 cast of W0 (split across DVE and Act)
            w0_f16 = []
            for kb in range(HB):
                t32, ksz = w0_f32[kb]
                tf = const.tile([128, 2 * HP], f16, name=f"w0f16_{kb}")
                nc.gpsimd.memset(tf, 0.0)
                nc.vector.tensor_scalar(out=tf[0:ksz, 0:H], in0=t32[0:ksz, 0:H], scalar1=0.0, scalar2=None, op0=Alu.add)
                nc.scalar.copy(tf[0:ksz, HP:HP + H], t32[0:ksz, H:2 * H])
                w0_f16.append(tf)

            # hT[hb]: [128 h, 256 n]  (PE transpose of fp16 tiles)
            ident = const.tile([128, 128], f16)
            make_identity(nc, ident)
            hT = [const.tile([128, N], f16, name=f"hT_{k}") for k in range(HB)]
            ncopy = 0

            def psum_copy(dst_sl, pst):
                nonlocal ncopy
                if ncopy % 2 == 0:
                    nc.vector.tensor_scalar(out=dst_sl, in0=pst, scalar1=0.0, scalar2=None, op0=Alu.add)
                else:
                    nc.scalar.copy(dst_sl, pst)
                ncopy += 1

            for hb in range(HB):
                for ib in range(NB):
                    pst = psT.tile([128, 128], f16, tag="tr")
                    nc.tensor.transpose(pst, h_f16[ib][:, hb * 128:(hb + 1) * 128], ident)
                    psum_copy(hT[hb][:, ib * 128:(ib + 1) * 128], pst)

            # W0aT/W0bT[hb]: [128 h, 384 k] via PE transpose, kb-major so
            # phase A on kb can start while kb+1 transposes are in flight
            w0aT = [const.tile([128, HP], f16, name=f"w0aT_{k}") for k in range(HB)]
            w0bT = [const.tile([128, HP], f16, name=f"w0bT_{k}") for k in range(HB)]
            aib = [const.tile([128, N], f32, name=f"aib_{k}") for k in range(HB)]
            ajT = [const.tile([128, N], f16, name=f"ajT_{k}") for k in range(HB)]

            def emit_w0T(kb):
                for half, dst in ((0, w0aT), (1, w0bT)):
                    for hb in range(HB):
                        pst = psT.tile([128, 128], f16, tag="tr")
                        nc.tensor.transpose(
                            pst,
                            w0_f16[kb][:, half * HP + hb * 128: half * HP + (hb + 1) * 128],
                            ident,
                        )
                        psum_copy(dst[hb][:, kb * 128:(kb + 1) * 128], pst)

            # ------- phase A: aib = aiT + b0 (fp16), ajT (fp16) ----------------
            def emit_phaseA(kb):
                for wT, dst, is_ai in ((w0aT, aib, True), (w0bT, ajT, False)):
                    ps = psA.tile([128, N], f32, tag="A")
                    for hb in range(HB):
                        nc.tensor.matmul(
                            ps,
                            lhsT=wT[hb][:, kb * 128:(kb + 1) * 128],
                            rhs=hT[hb],
                            start=(hb == 0),
                            stop=(hb == HB - 1),
                        )
                    if is_ai:
                        nc.vector.tensor_scalar(
                            out=dst[kb], in0=ps, scalar1=b0c[kb], scalar2=None,
                            op0=Alu.add)
                    else:
                        nc.scalar.copy(dst[kb], ps)

            for kb in range(HB):
                emit_w0T(kb)
                emit_phaseA(kb)

            # Tail-pair setup: k-block 2 has only 44 real rows, so two queries'
            # tails share one op/matmul (rows 0:44 = query i, 64:108 = query
            # i+1 via a column-shifted bias layout).
            ajT_tail2 = const.tile([128, N], f16)
            aib_tail2 = const.tile([128, N], f32)
            nc.gpsimd.memset(ajT_tail2, 0.0)
            nc.gpsimd.memset(aib_tail2, 0.0)
            nc.scalar.copy(ajT_tail2[0:KT, :], ajT[2][0:KT, :])
            nc.scalar.copy(ajT_tail2[64:64 + KT, :], ajT[2][0:KT, :])
            nc.scalar.copy(aib_tail2[0:KT, :], aib[2][0:KT, :])
            nc.scalar.copy(aib_tail2[64:64 + KT, 0:N - 1], aib[2][0:KT, 1:N])

            # ------- phase B: p[i, j] = w1 . relu(ajT[:, j] + aib[:, i]) -------
            # eIJ[ib][i % 128, j] = exp(p[i, j]) for i in block ib
            eIJ = [const.tile([128, N], f16, name=f"eIJ_{ib}") for ib in range(NB)]
            Pg = None
            for i0 in range(0, N, 2):
                g, m0 = divmod(i0, GROUP)
                slab, mm = divmod(m0, 32)
                if m0 == 0:
                    Pg = psP.tile([128, N], f32, tag="P")
                act_extra = (i0 // 2) % 3 != 0
                tt = tpool.tile([128, 4 * N], f16, tag="T")
                ttt = tpool.tile([128, N], f16, tag="Tt")
                for q in range(2):
                    for kb in range(2):
                        out_sl = tt[:, (q * 2 + kb) * N:(q * 2 + kb + 1) * N]
                        bias = aib[kb][:, i0 + q:i0 + q + 1]
                        if act_extra and q == 1 and kb == 1:
                            nc.scalar.activation(out=out_sl, in_=ajT[kb], func=Relu,
                                                 bias=bias, scale=1.0)
                        else:
                            nc.vector.tensor_scalar(
                                out=out_sl, in0=ajT[kb],
                                scalar1=bias, scalar2=0.0,
                                op0=Alu.add, op1=Alu.max)
                nc.scalar.activation(out=ttt, in_=ajT_tail2, func=Relu,
                                     bias=aib_tail2[:, i0:i0 + 1], scale=1.0)
                out_sl = Pg[32 * slab:32 * (slab + 1), :]
                tpos = (0, 32 * slab)
                first = (mm == 0)
                for q in range(2):
                    m = mm + q
                    for kb in range(2):
                        nc.tensor.matmul(
                            out_sl,
                            lhsT=Z[kb][:, 31 - m:63 - m],
                            rhs=tt[:, (q * 2 + kb) * N:(q * 2 + kb + 1) * N],
                            start=first,
                            stop=False,
                            tile_position=tpos,
                        )
                        first = False
                nc.tensor.matmul(
                    out_sl,
                    lhsT=ZT[:, 31 - mm:63 - mm],
                    rhs=ttt,
                    start=False,
                    stop=(mm == 30),
                    tile_position=tpos,
                )
                if m0 == GROUP - 2:
                    nc.scalar.activation(out=eIJ[g], in_=Pg, func=Exp)

            # ---------------- softmax + output --------------------------------
            # eT[jb]: [128 j, 256 i] — PE transpose of exp(p) blocks
            eT = [const.tile([128, N], f16, name=f"eT_{jb}") for jb in range(NB)]
            for ib in range(NB):
                for jb in range(NB):
                    pst = psT.tile([128, 128], f16, tag="tr")
                    nc.tensor.transpose(pst, eIJ[ib][:, jb * 128:(jb + 1) * 128], ident)
                    psum_copy(eT[jb][:, ib * 128:(ib + 1) * 128], pst)

            # final: one matmul group per ib gives u = e^T.T @ h AND the row
            # sum s in the appended ones column; y = u * (1/s) per partition
            for ib in range(NB):
                pso = psO.tile([128, H + 1], f32, tag="O")
                for jb in range(NB):
                    nc.tensor.matmul(
                        pso,
                        lhsT=eT[jb][:, ib * 128:(ib + 1) * 128],
                        rhs=h_f16[jb][:, 0:H + 1],
                        start=(jb == 0),
                        stop=(jb == NB - 1),
                    )
                rcol = work.tile([128, 1], f32, tag=f"rcol{ib}")
                nc.vector.reciprocal(rcol, pso[:, H:H + 1])
                yt = work.tile([128, H], f32, tag="y")
                nc.vector.tensor_scalar(
                    out=yt, in0=pso[:, 0:H], scalar1=rcol, scalar2=None, op0=Alu.mult,
                )
                nc.sync.dma_start(out=y_out[ib * 128:(ib + 1) * 128, :], in_=yt)
    return nc


def build_nc():
    nc = bacc.Bacc("TRN2", target_bir_lowering=False, debug=False, num_devices=B)
    _emit(nc)
    nc.compile()
    return nc


def _get_nc():
    if "nc" not in _CACHE:
        _CACHE["nc"] = build_nc()
    return _CACHE["nc"]


def kernel(h_prev, W0, b0, W1, b1, **_ignored):
    del b1  # softmax is invariant to the scalar output bias
    h_prev = np.asarray(h_prev, np.float32)
    W0 = np.asarray(W0, np.float32)
    b0 = np.asarray(b0, np.float32).reshape(H)
    w1 = np.asarray(W1, np.float32).reshape(H)
    assert h_prev.shape == (B, N, H), h_prev.shape

    nc = _get_nc()
    in_maps = [
        {"h": np.ascontiguousarray(h_prev[c]), "w0": W0, "b0": b0, "w1": w1}
        for c in range(B)
    ]
    res = run_bass_kernel_spmd(nc, in_maps, core_ids=list(range(B)))
    return np.stack([res.results[c]["y"] for c in range(B)], axis=0).astype(np.float32)


# revision 13
# speedup vs baseline: 4.1935x; 1.0346x over previous
"""EvidenceLevelAttention (additive attention GNN message passing) on 8 trn2 cores.

Math per batch b (B=8, N=256, H=300):
    ai = h @ W0a.T ; aj = h @ W0b.T                     (W0a = W0[:, :H], W0b = W0[:, H:])
    p[i, j] = w1 . relu(ai[i] + aj[j] + b0)  (+ b1, dropped: softmax shift-invariant)
    a = softmax(p, axis=-1) ;  y = a @ h

Data-parallel: core c computes batch c. Heavy math in fp16 with fp32 PSUM
accumulation.

Pairwise phase: hidden dim k (300 -> padded 384 = 3x128) sits on partitions so
the per-i bias is a per-partition scalar and relu(ajT + aib[:, i]) is ONE fused
tensor_scalar(add, max) / activation(Relu, bias) per (i, k-block). The ops are
split ~2:1 between DVE (~197ns/op) and the Act engine (~400ns/op) so both
engines drain together; GpSimd takes ~4us for this op shape and only gets
memsets. DMA XBAR transposes are avoided too: each dma_start_transpose costs
>1us of issue time on its queue.

The w1 contraction runs the relu tiles through the PE as the MOVING operand:
the stationary is a [128, 32] sliding window of a zero tile with the w1
k-block at one column, so query i's dot products land on psum partition
i mod 128 (32-row slab selected via the out AP / tile_position, column within
the slab by the window shift). p accumulates directly in [i row, j col] layout,
one 32-query slab at a time, inside two [128, 256] psum group tiles. The 32-col
stationary keeps LDWEIGHTS short enough to hide under the 256-col stream
(~128ns/matmul). Tails of a query pair (k rows 256:300) share one matmul via a
two-column stationary window.

Softmax needs no max-subtraction (p is O(1)); exp runs on Act straight out of
PSUM, row sums come from an appended ones column in the final eT.T @ h matmul,
and 1/s scales the output per partition.
"""

import numpy as np

import concourse.bass as bass
import concourse.mybir as mybir
import concourse.tile as tile
from concourse import bacc
from concourse.bass_utils import run_bass_kernel_spmd
from concourse.masks import make_identity

B, N, H = 8, 256, 300
HB = 3          # hidden-dim blocks of 128
HP = HB * 128   # padded hidden dim
NB = 2          # row blocks of 128
F32 = mybir.dt.float32
F16 = mybir.dt.float16
GROUP = 128     # queries per psum accumulation group
KT = H - 2 * 128  # 44 tail rows of the hidden dim
T_BUFS = 8

_CACHE = {}


def _emit(nc):
    f32, f16 = F32, F16
    Alu = mybir.AluOpType
    Exp = mybir.ActivationFunctionType.Exp
    Relu = mybir.ActivationFunctionType.Relu
    Copy = mybir.ActivationFunctionType.Copy

    h_in = nc.dram_tensor("h", [N, H], f32, kind="ExternalInput")
    w0_in = nc.dram_tensor("w0", [H, 2 * H], f32, kind="ExternalInput")
    b0_in = nc.dram_tensor("b0", [H], f32, kind="ExternalInput")
    w1_in = nc.dram_tensor("w1", [H], f32, kind="ExternalInput")
    y_out = nc.dram_tensor("y", [N, H], f32, kind="ExternalOutput")

    with tile.TileContext(nc) as tc:
        with (
            tc.tile_pool(name="const", bufs=1) as const,
            tc.tile_pool(name="work", bufs=2) as work,
            tc.tile_pool(name="tpool", bufs=T_BUFS) as tpool,
            tc.tile_pool(name="psA", bufs=2, space="PSUM") as psA,
            tc.tile_pool(name="psT", bufs=3, space="PSUM") as psT,
            tc.tile_pool(name="psP", bufs=2, space="PSUM") as psP,
            tc.tile_pool(name="psO", bufs=1, space="PSUM") as psO,
        ):
            # ---------------- phase 0: loads, casts, transposes ----------------
            # W0 first: it heads the longest dependency chain (cast ->
            # transpose -> phase A). k-blocked rows, cols [W0a | pad | W0b | pad].
            w0_f32 = []
            for kb in range(HB):
                k0 = kb * 128
                ksz = min(H, k0 + 128) - k0
                t32 = work.tile([128, 2 * H], f32, tag=f"w0scratch{kb}")
                nc.sync.dma_start(out=t32[0:ksz, :], in_=w0_in[k0:k0 + ksz, :])
                w0_f32.append((t32, ksz))

            # h rows, fp32 then fp16 (k-padded with zeros)
            h_f32 = [const.tile([128, H], f32, name=f"h_f32_{k}") for k in range(NB)]
            h_f16 = [const.tile([128, HP], f16, name=f"h_f16_{k}") for k in range(NB)]
            for ib in range(NB):
                nc.sync.dma_start(out=h_f32[ib], in_=h_in[ib * 128:(ib + 1) * 128, :])
                nc.gpsimd.memset(h_f16[ib][:, H:HP], 0.0)
                nc.gpsimd.memset(h_f16[ib][:, H:H + 1], 1.0)  # ones col for fused row-sum
                nc.vector.tensor_scalar(out=h_f16[ib][:, 0:H], in0=h_f32[ib], scalar1=0.0, scalar2=None, op0=Alu.add)

            # b0 (fp32) and w1 (fp16) as per-partition columns over k-blocks
            b0c = [const.tile([128, 1], f32, name=f"b0c_{k}") for k in range(HB)]
            w1c = [const.tile([128, 1], f16, name=f"w1c_{k}") for k in range(HB)]
            for kb in range(HB):
                k0 = kb * 128
                ksz = min(H, k0 + 128) - k0
                w1f = work.tile([128, 1], f32, tag="w1scratch")
                nc.gpsimd.memset(b0c[kb], 0.0)
                nc.gpsimd.memset(w1c[kb], 0.0)
                nc.sync.dma_start(out=b0c[kb][0:ksz, 0:1], in_=b0_in[k0:k0 + ksz])
                nc.sync.dma_start(out=w1f[0:ksz, 0:1], in_=w1_in[k0:k0 + ksz])
                nc.vector.tensor_scalar(out=w1c[kb][0:ksz, :], in0=w1f[0:ksz, :], scalar1=0.0, scalar2=None, op0=Alu.add)

            # Sliding-window stationaries for the w1 contraction: Z[kb] is zero
            # except col 31 = w1 k-block; lhsT = Z[kb][:, 31-mm : 63-mm] puts
            # w1 at stationary column mm. ZT packs both tail halves of a query
            # pair: col 31 rows 0:44 and col 32 rows 64:108 (matching the
            # packed tail tile layout), so one matmul does both queries' tails.
            Z = [const.tile([128, 63], f16, name=f"Z_{kb}") for kb in range(2)]
            for kb in range(2):
                nc.gpsimd.memset(Z[kb], 0.0)
                nc.vector.tensor_scalar(out=Z[kb][:, 31:32], in0=w1c[kb], scalar1=0.0, scalar2=None, op0=Alu.add)
            ZT = const.tile([128, 63], f16, name="ZT")
            nc.gpsimd.memset(ZT, 0.0)
            nc.vector.tensor_scalar(out=ZT[0:KT, 31:32], in0=w1c[2][0:KT, :], scalar1=0.0, scalar2=None, op0=Alu.add)
            nc.vector.tensor_scalar(out=ZT[64:64 + KT, 32:33], in0=w1c[2][0:KT, :], scalar1=0.0, scalar2=None, op0=Alu.add)

            # fp16 cast of W0 (split across DVE and Act)
            w0_f16 = []
            for kb in range(HB):
                t32, ksz = w0_f32[kb]
                tf = const.tile([128, 2 * HP], f16, name=f"w0f16_{kb}")
                nc.gpsimd.memset(tf, 0.0)
                nc.vector.tensor_scalar(out=tf[0:ksz, 0:H], in0=t32[0:ksz, 0:H], scalar1=0.0, scalar2=None, op0=Alu.add)
                nc.scalar.copy(tf[0:ksz, HP:HP + H], t32[0:ksz, H:2 * H])
                w0_f16.append(tf)

            # hT[hb]: [128 h, 256 n]  (PE transpose of fp16 tiles)
            ident = const.tile([128, 128], f16)
            make_identity(nc, ident)
            hT = [const.tile([128, N], f16, name=f"hT_{k}") for k in range(HB)]
            ncopy = 0

            def psum_copy(dst_sl, pst):
                nonlocal ncopy
                if ncopy % 2 == 0:
                    nc.vector.tensor_scalar(out=dst_sl, in0=pst, scalar1=0.0, scalar2=None, op0=Alu.add)
                else:
                    nc.scalar.copy(dst_sl, pst)
                ncopy += 1

            for hb in range(HB):
                for ib in range(NB):
                    pst = psT.tile([128, 128], f16, tag="tr")
                    nc.tensor.transpose(pst, h_f16[ib][:, hb * 128:(hb + 1) * 128], ident)
                    psum_copy(hT[hb][:, ib * 128:(ib + 1) * 128], pst)

            # W0aT/W0bT[hb]: [128 h, 384 k] via PE transpose, kb-major so
            # phase A on kb can start while kb+1 transposes are in flight
            w0aT = [const.tile([128, HP], f16, name=f"w0aT_{k}") for k in range(HB)]
            w0bT = [const.tile([128, HP], f16, name=f"w0bT_{k}") for k in range(HB)]
            aib = [const.tile([128, N], f32, name=f"aib_{k}") for k in range(HB)]
            ajT = [const.tile([128, N], f16, name=f"ajT_{k}") for k in range(HB)]

            def emit_w0T(kb):
                for half, dst in ((0, w0aT), (1, w0bT)):
                    for hb in range(HB):
                        pst = psT.tile([128, 128], f16, tag="tr")
                        nc.tensor.transpose(
                            pst,
                            w0_f16[kb][:, half * HP + hb * 128: half * HP + (hb + 1) * 128],
                            ident,
                        )
                        psum_copy(dst[hb][:, kb * 128:(kb + 1) * 128], pst)

            # ------- phase A: aib = aiT + b0 (fp16), ajT (fp16) ----------------
            def emit_phaseA(kb):
                for wT, dst, is_ai in ((w0aT, aib, True), (w0bT, ajT, False)):
                    ps = psA.tile([128, N], f32, tag="A")
                    for hb in range(HB):
                        nc.tensor.matmul(
                            ps,
                            lhsT=wT[hb][:, kb * 128:(kb + 1) * 128],
                            rhs=hT[hb],
                            start=(hb == 0),
                            stop=(hb == HB - 1),
                        )
                    if is_ai:
                        nc.vector.tensor_scalar(
                            out=dst[kb], in0=ps, scalar1=b0c[kb], scalar2=None,
                            op0=Alu.add)
                    else:
                        nc.scalar.copy(dst[kb], ps)

            for kb in range(HB):
                emit_w0T(kb)
                emit_phaseA(kb)

            # Tail-pair setup: k-block 2 has only 44 real rows, so two queries'
            # tails share one op/matmul (rows 0:44 = query i, 64:108 = query
            # i+1 via a column-shifted bias layout).
            ajT_tail2 = const.tile([128, N], f16)
            aib_tail2 = const.tile([128, N], f32)
            nc.gpsimd.memset(ajT_tail2, 0.0)
            nc.gpsimd.memset(aib_tail2, 0.0)
            nc.scalar.copy(ajT_tail2[0:KT, :], ajT[2][0:KT, :])
            nc.scalar.copy(ajT_tail2[64:64 + KT, :], ajT[2][0:KT, :])
            nc.scalar.copy(aib_tail2[0:KT, :], aib[2][0:KT, :])
            nc.scalar.copy(aib_tail2[64:64 + KT, 0:N - 1], aib[2][0:KT, 1:N])

            # ------- phase B: p[i, j] = w1 . relu(ajT[:, j] + aib[:, i]) -------
            # eIJ[ib][i % 128, j] = exp(p[i, j]) for i in block ib
            eIJ = [const.tile([128, N], f16, name=f"eIJ_{ib}") for ib in range(NB)]
            Pg = None
            for i0 in range(0, N, 2):
                g, m0 = divmod(i0, GROUP)
                slab, mm = divmod(m0, 32)
                if m0 == 0:
                    Pg = psP.tile([128, N], f32, tag="P")
                act_extra = (i0 // 2) % 8 in (1, 3, 4, 6, 7)
                tt = tpool.tile([128, 4 * N], f16, tag="T")
                ttt = tpool.tile([128, N], f16, tag="Tt")
                for q in range(2):
                    for kb in range(2):
                        out_sl = tt[:, (q * 2 + kb) * N:(q * 2 + kb + 1) * N]
                        bias = aib[kb][:, i0 + q:i0 + q + 1]
                        if act_extra and q == 1 and kb == 1:
                            nc.scalar.activation(out=out_sl, in_=ajT[kb], func=Relu,
                                                 bias=bias, scale=1.0)
                        else:
                            nc.vector.tensor_scalar(
                                out=out_sl, in0=ajT[kb],
                                scalar1=bias, scalar2=0.0,
                                op0=Alu.add, op1=Alu.max)
                nc.scalar.activation(out=ttt, in_=ajT_tail2, func=Relu,
                                     bias=aib_tail2[:, i0:i0 + 1], scale=1.0)
                out_sl = Pg[32 * slab:32 * (slab + 1), :]
                tpos = (0, 32 * slab)
                first = (mm == 0)
                for q in range(2):
                    m = mm + q
                    for kb in range(2):
                        nc.tensor.matmul(
                            out_sl,
                            lhsT=Z[kb][:, 31 - m:63 - m],
                            rhs=tt[:, (q * 2 + kb) * N:(q * 2 + kb + 1) * N],
                            start=first,
                            stop=False,
                            tile_position=tpos,
                        )
                        first = False
                nc.tensor.matmul(
                    out_sl,
                    lhsT=ZT[:, 31 - mm:63 - mm],
                    rhs=ttt,
                    start=False,
                    stop=(mm == 30),
                    tile_position=tpos,
                )
                if m0 == GROUP - 2:
                    # group done: exp, transpose its exp(p) blocks, and emit
                    # this row-block's output while later groups keep running
                    nc.scalar.activation(out=eIJ[g], in_=Pg, func=Exp)
                    ib = g
                    eTb = [tpool.tile([128, 128], f16, tag=f"eT{jb}", name=f"eTb_{g}_{jb}") for jb in range(NB)]
                    for jb in range(NB):
                        pst = psT.tile([128, 128], f16, tag="tr")
                        nc.tensor.transpose(pst, eIJ[ib][:, jb * 128:(jb + 1) * 128], ident)
                        psum_copy(eTb[jb], pst)
                    # u = e^T.T @ h with the appended ones column giving the
                    # row sum s; y = u * (1/s) per partition
                    pso = psO.tile([128, H + 1], f32, tag="O")
                    for jb in range(NB):
                        nc.tensor.matmul(
                            pso,
                            lhsT=eTb[jb],
                            rhs=h_f16[jb][:, 0:H + 1],
                            start=(jb == 0),
                            stop=(jb == NB - 1),
                        )
                    rcol = work.tile([128, 1], f32, tag=f"rcol{ib}")
                    nc.vector.reciprocal(rcol, pso[:, H:H + 1])
                    yt = work.tile([128, H], f32, tag=f"y{ib}")
                    nc.vector.tensor_scalar(
                        out=yt, in0=pso[:, 0:H], scalar1=rcol, scalar2=None, op0=Alu.mult,
                    )
                    nc.sync.dma_start(out=y_out[ib * 128:(ib + 1) * 128, :], in_=yt)
    return nc


def build_nc():
    nc = bacc.Bacc("TRN2", target_bir_lowering=False, debug=False, num_devices=B)
    _emit(nc)
    nc.compile()
    return nc


def _get_nc():
    if "nc" not in _CACHE:
        _CACHE["nc"] = build_nc()
    return _CACHE["nc"]


def kernel(h_prev, W0, b0, W1, b1, **_ignored):
    del b1  # softmax is invariant to the scalar output bias
    h_prev = np.asarray(h_prev, np.float32)
    W0 = np.asarray(W0, np.float32)
    b0 = np.asarray(b0, np.float32).reshape(H)
    w1 = np.asarray(W1, np.float32).reshape(H)
    assert h_prev.shape == (B, N, H), h_prev.shape

    nc = _get_nc()
    in_maps = [
        {"h": np.ascontiguousarray(h_prev[c]), "w0": W0, "b0": b0, "w1": w1}
        for c in range(B)
    ]
    res = run_bass_kernel_spmd(nc, in_maps, core_ids=list(range(B)))
    return np.stack([res.results[c]["y"] for c in range(B)], axis=0).astype(np.float32)
